# revision 1
# baseline (speedup 1.0000x reference)
"""CrossLayerTranscoderSystem kernel for 8x Trainium2 NeuronCores.

Math (reference):
  for each layer l: pre = x[l] @ We[l].T + be[l]            [B*S, H]
                    f   = pre * (pre > thr[l])              (JumpReLU)
                    feats[l] = top-64-masked f              (dense, 64 nonzero/row)
  for each layer t: out[t] = bo[t] + sum_{s<=t} feats[s] @ Wd[s,t].T

Strategy: data-parallel over the 1024 tokens (128 tokens per core, exactly one
128-partition tile). Weights replicated per core. Encoder runs in fp32 (exact
top-k selection); top-64 via 8 rounds of DVE max/match_replace (threshold
masking); decode streams host-pre-transposed bf16 weights. Output bias is
added on host (it is a per-output-feature broadcast; cheap in numpy).
"""

import numpy as np
import ml_dtypes

L, D, H, O, K = 6, 768, 4096, 768, 64
NTOK = 1024  # B*S = 4*256
NCORES = 8
T = NTOK // NCORES  # 128 tokens per core
B, S = 4, 256
KD = D // 128       # 6 contraction k-tiles for the encoder
HC = H // 512       # 8 psum-width chunks of the hidden dim
NJ = H // 128       # 32 decode contraction k-tiles
PAIRS = [(s, t) for t in range(L) for s in range(t + 1)]  # 21 used (s,t) pairs
NPAIRS = len(PAIRS)
WD_CHUNK_J = 8      # j-tiles per decode weight DMA (8*128*768*2B = 1.5 MiB)
NEG_SENTINEL = -1e30

_cache = {}


def _build_program(has_enc_bias: bool, has_threshold: bool, *, wd_chunk_j=WD_CHUNK_J,
                   we_bufs=3, wd_bufs=3, x_bufs=2, f_bufs=3, pse_bufs=4,
                   pst_bufs=2, psd_bufs=1, enc_halves=2, balanced=False, topk_rounds=K//8, dec_n=None, enc_n=None, fast_mask=True, work_bufs=1, osb_bufs=2, split_topk=False, wd_eng='sync', we_eng='scalar', reps=1):
    import concourse.bacc as bacc
    import concourse.bass as bass
    import concourse.mybir as mybir
    from concourse import tile
    from concourse.masks import make_identity

    dt = mybir.dt
    f32, bf16 = dt.float32, dt.bfloat16
    if has_enc_bias or has_threshold:
        we_bufs = min(we_bufs, 2)  # keep SBUF in budget on the general path

    # balanced decode schedule: spread the 21 (s,t) blocks across the 6
    # encoder slots so the decoder-weight DMA is ~constant per slot
    sched = []
    if balanced:
        quota = [3, 4, 3, 4, 3, 4]
        remaining = [(t_, s_) for t_ in range(L) for s_ in range(t_ + 1)]
        for sl in range(L):
            take = []
            for ts in remaining:
                if len(take) >= quota[sl]:
                    break
                if ts[1] <= sl:
                    take.append(ts)
            for ts in take:
                remaining.remove(ts)
            sched.append(take)
        assert not remaining

    nc = bacc.Bacc("TRN2", target_bir_lowering=False, debug=False)

    xTp = nc.dram_tensor("xTp", [L, 128, KD * T], f32, kind="ExternalInput")
    weTp = nc.dram_tensor("weTp", [L, KD, 128, H], f32, kind="ExternalInput")
    wdTp = nc.dram_tensor("wdTp", [NPAIRS, 128, NJ * O], bf16, kind="ExternalInput")
    if has_enc_bias:
        be_d = nc.dram_tensor("be", [L, H], f32, kind="ExternalInput")
    if has_threshold:
        thr_d = nc.dram_tensor("thr", [L, H], f32, kind="ExternalInput")
    out_d = nc.dram_tensor("out", [L, T, O], f32, kind="ExternalOutput")

    with tile.TileContext(nc) as tc:
        with (
            tc.tile_pool(name="const", bufs=1) as const_pool,
            tc.tile_pool(name="xin", bufs=x_bufs) as x_pool,
            tc.tile_pool(name="we", bufs=we_bufs) as we_pool,
            tc.tile_pool(name="wd", bufs=wd_bufs) as wd_pool,
            tc.tile_pool(name="fsb", bufs=f_bufs) as f_pool,
            tc.tile_pool(name="work", bufs=work_bufs) as work_pool,
            tc.tile_pool(name="feats", bufs=1) as feats_pool,
            tc.tile_pool(name="featsT", bufs=1) as featsT_pool,
            tc.tile_pool(name="t8", bufs=2) as t8_pool,
            tc.tile_pool(name="osb", bufs=osb_bufs) as o_pool,
            tc.tile_pool(name="pse", bufs=pse_bufs, space="PSUM") as psum_e,
            tc.tile_pool(name="pst", bufs=pst_bufs, space="PSUM") as psum_t,
            tc.tile_pool(name="psd", bufs=psd_bufs, space="PSUM") as psum_d,
        ):
            ident = const_pool.tile([128, 128], f32, tag="ident")
            make_identity(nc, ident[:])
            if has_enc_bias:
                ones_t = const_pool.tile([1, T], f32, tag="ones")
                nc.vector.memset(ones_t[:], 1.0)

            if balanced:
                out_sb = [
                    o_pool.tile([128, O], f32, tag=f"oacc{t_}", name=f"oacc{t_}",
                                bufs=1)
                    for t_ in range(L)
                ]
            # per-layer transposed masked features, bf16 [128, NJ*128]
            featsT = [
                featsT_pool.tile([128, H], bf16, tag=f"featsT{l}", name=f"featsT{l}")
                for l in range(L)
            ]

            for rep_t in range(reps * L):
                t = rep_t % L
                # ---------------- encoder for layer l = t ----------------
                l = t
                x_t = x_pool.tile([128, KD * T], f32, tag="x")
                nc.sync.dma_start(x_t[:], xTp[l])

                if has_enc_bias:
                    be_t = x_pool.tile([1, H], f32, tag="be", bufs=1)
                    nc.sync.dma_start(be_t[:], be_d[l : l + 1, :])

                f_sb = f_pool.tile([128, H], f32, tag="f")
                nq = HC // enc_halves
                hw_ = H // enc_halves
                for half in range(enc_halves):
                    ps = [
                        psum_e.tile([128, 512], f32, tag="pse", name=f"pse{q}")
                        for q in range(nq)
                    ]
                    for kd in range(KD):
                        w_t = we_pool.tile([128, hw_], f32, tag="we")
                        getattr(nc, we_eng).dma_start(
                            w_t[:], weTp[l, kd, :, half * hw_ : (half + 1) * hw_]
                        )
                        for q in range(nq):
                            en = 512 if enc_n is None else enc_n
                            nc.tensor.matmul(
                                ps[q][:, :en],
                                x_t[:, kd * T : (kd + 1) * T],
                                w_t[:, q * 512 : q * 512 + en],
                                start=(kd == 0),
                                stop=(kd == KD - 1 and not has_enc_bias),
                            )
                    if has_enc_bias:
                        for q in range(nq):
                            hc = half * nq + q
                            nc.tensor.matmul(
                                ps[q][:],
                                ones_t[:],
                                be_t[:, hc * 512 : (hc + 1) * 512],
                                start=False,
                                stop=True,
                            )
                    for q in range(nq):
                        hc = half * nq + q
                        # drain psum -> f; fast path fuses the JumpReLU (thr==0)
                        nc.scalar.activation(
                            f_sb[:, hc * 512 : (hc + 1) * 512],
                            ps[q][:],
                            mybir.ActivationFunctionType.Copy
                            if has_threshold
                            else mybir.ActivationFunctionType.Relu,
                        )

                work = work_pool.tile([128, H], f32, tag="work")
                if has_threshold:
                    # general JumpReLU: f = z * (z > thr); thr varies along the
                    # free axis so replicate it across partitions, reusing the
                    # top-k scratch tile before the top-k rounds need it
                    nc.sync.dma_start(work[:1, :], thr_d[l : l + 1, :])
                    nc.gpsimd.partition_broadcast(work[:], work[:1, :])
                    nc.vector.tensor_tensor(
                        out=work[:], in0=f_sb[:], in1=work[:],
                        op=mybir.AluOpType.is_gt,
                    )
                    nc.vector.tensor_mul(f_sb[:], f_sb[:], work[:])

                # ---------------- top-64 masking ----------------
                if split_topk and fast_mask and topk_rounds == 8:
                    # per-half top-64 (starts as soon as that half's encoder
                    # drains land), then merge the 2x64 candidates
                    cand = t8_pool.tile([128, 128], f32, tag="cand", bufs=2)
                    for hh in range(2):
                        sl = slice(hh * 2048, (hh + 1) * 2048)
                        hsrc = f_sb[:, sl]
                        for r in range(8):
                            slot = cand[:, (hh * 8 + r) * 8 : (hh * 8 + r + 1) * 8]
                            nc.vector.max(out=slot, in_=hsrc)
                            if r < 7:
                                nc.vector.match_replace(
                                    out=work[:, sl], in_to_replace=slot,
                                    in_values=hsrc, imm_value=NEG_SENTINEL,
                                )
                                hsrc = work[:, sl]
                    csrc = cand
                    cwork = t8_pool.tile([128, 128], f32, tag="cwork", bufs=2)
                    for r in range(8):
                        t8m = t8_pool.tile([128, 8], f32, tag="t8")
                        nc.vector.max(out=t8m[:], in_=csrc[:])
                        last_t8 = t8m
                        if r < 7:
                            nc.vector.match_replace(
                                out=cwork[:], in_to_replace=t8m[:],
                                in_values=csrc[:], imm_value=NEG_SENTINEL,
                            )
                            csrc = cwork
                    feats = f_sb
                    nc.vector.scalar_tensor_tensor(
                        out=feats[:], in0=f_sb[:], scalar=last_t8[:, 7:8],
                        in1=f_sb[:], op0=mybir.AluOpType.is_ge,
                        op1=mybir.AluOpType.mult,
                    )
                else:
                  src = f_sb
                  last_t8 = None
                  for r in range(topk_rounds):
                      t8 = t8_pool.tile([128, 8], f32, tag="t8")
                      nc.vector.max(out=t8[:], in_=src[:])
                      last_t8 = t8
                      if fast_mask and r == topk_rounds - 1:
                          break  # tau is t8[:,7]; no need to zap the last 8
                      nc.vector.match_replace(
                          out=work[:], in_to_replace=t8[:], in_values=src[:],
                          imm_value=NEG_SENTINEL,
                      )
                      src = work
                  if fast_mask:
                    feats = f_sb  # mask in place; raw f values are dead after this
                    nc.vector.scalar_tensor_tensor(
                        out=feats[:],
                        in0=f_sb[:],
                        scalar=last_t8[:, 7:8],
                        in1=f_sb[:],
                        op0=mybir.AluOpType.is_ge,
                        op1=mybir.AluOpType.mult,
                    )
                  elif True:
                      if balanced:
                          feats = f_sb
                      else:
                          feats = feats_pool.tile([128, H], f32, tag="feats")
                      # feats = (work <= sentinel? original value : 0)
                      nc.vector.scalar_tensor_tensor(
                          out=feats[:],
                          in0=work[:],
                          scalar=NEG_SENTINEL * 0.5,
                          in1=f_sb[:],
                          op0=mybir.AluOpType.is_le,
                          op1=mybir.AluOpType.mult,
                      )

                # ---------------- transpose feats -> featsT[l] (bf16) ----------------
                for j in range(NJ):
                    pt = psum_t.tile([128, 128], f32, tag="pst")
                    nc.tensor.transpose(
                        pt[:], feats[:, j * 128 : (j + 1) * 128], ident[:]
                    )
                    nc.scalar.activation(
                        featsT[l][:, j * 128 : (j + 1) * 128],
                        pt[:],
                        mybir.ActivationFunctionType.Copy,
                    )

                # ---------------- decode ----------------
                if balanced:
                    for (tt, ss) in sched[t]:
                        p = PAIRS.index((ss, tt))
                        ps0 = psum_d.tile([128, 512], f32, tag="psd0")
                        ps1 = psum_d.tile([128, 256], f32, tag="psd1")
                        i = 0
                        for jc in range(NJ // wd_chunk_j):
                            wdt = wd_pool.tile([128, wd_chunk_j * O], bf16,
                                               tag="wd", name="wdt")
                            getattr(nc, wd_eng).dma_start(
                                wdt[:],
                                wdTp[p, :, jc * wd_chunk_j * O
                                     : (jc + 1) * wd_chunk_j * O],
                            )
                            for jj in range(wd_chunk_j):
                                j = jc * wd_chunk_j + jj
                                lh = featsT[ss][:, j * 128 : (j + 1) * 128]
                                nc.tensor.matmul(ps0[:], lh,
                                                 wdt[:, jj * O : jj * O + 512],
                                                 start=(i == 0), stop=(i == NJ - 1))
                                nc.tensor.matmul(ps1[:], lh,
                                                 wdt[:, jj * O + 512 : (jj + 1) * O],
                                                 start=(i == 0), stop=(i == NJ - 1))
                                i += 1
                        if ss == 0:  # first block of chain tt
                            nc.scalar.activation(out_sb[tt][:, 0:512], ps0[:],
                                                 mybir.ActivationFunctionType.Copy)
                            nc.scalar.activation(out_sb[tt][:, 512:O], ps1[:],
                                                 mybir.ActivationFunctionType.Copy)
                        else:
                            nc.vector.tensor_tensor(
                                out=out_sb[tt][:, 0:512], in0=out_sb[tt][:, 0:512],
                                in1=ps0[:], op=mybir.AluOpType.add)
                            nc.vector.tensor_tensor(
                                out=out_sb[tt][:, 512:O], in0=out_sb[tt][:, 512:O],
                                in1=ps1[:], op=mybir.AluOpType.add)
                        if ss == tt:  # last block of chain tt
                            nc.sync.dma_start(out_d[tt], out_sb[tt][:])
                    continue
                ps0 = psum_d.tile([128, 512], f32, tag="psd0")
                ps1 = psum_d.tile([128, 256], f32, tag="psd1")
                nmm = (t + 1) * NJ
                i = 0
                for s in range(t + 1):
                    p = PAIRS.index((s, t))
                    for jc in range(NJ // wd_chunk_j):
                        wdt = wd_pool.tile([128, wd_chunk_j * O], bf16, tag="wd")
                        getattr(nc, wd_eng).dma_start(
                            wdt[:],
                            wdTp[p, :, jc * wd_chunk_j * O : (jc + 1) * wd_chunk_j * O],
                        )
                        for jj in range(wd_chunk_j):
                            j = jc * wd_chunk_j + jj
                            lh = featsT[s][:, j * 128 : (j + 1) * 128]
                            dn0 = 512 if dec_n is None else dec_n
                            dn1 = 256 if dec_n is None else dec_n
                            nc.tensor.matmul(
                                ps0[:, :dn0],
                                lh,
                                wdt[:, jj * O : jj * O + dn0],
                                start=(i == 0),
                                stop=(i == nmm - 1),
                            )
                            nc.tensor.matmul(
                                ps1[:, :dn1],
                                lh,
                                wdt[:, jj * O + 512 : jj * O + 512 + dn1],
                                start=(i == 0),
                                stop=(i == nmm - 1),
                            )
                            i += 1
                o_sb = o_pool.tile([128, O], f32, tag="osb")
                nc.scalar.activation(
                    o_sb[:, 0:512], ps0[:], mybir.ActivationFunctionType.Copy
                )
                nc.scalar.activation(
                    o_sb[:, 512:O], ps1[:], mybir.ActivationFunctionType.Copy
                )
                nc.sync.dma_start(out_d[t], o_sb[:])

    nc.compile()
    return nc


def _prep_inputs(layer_inputs, enc_weight, enc_bias, threshold, dec_weight):
    """Host-side repack into DMA-friendly layouts."""
    bf = ml_dtypes.bfloat16
    x = np.ascontiguousarray(layer_inputs.reshape(L, NTOK, D))

    # xTp per core: [L, 128, KD*T]; [p, kd*T+tok] = x[l, tok_c, kd*128+p]
    xTp_cores = []
    for c in range(NCORES):
        xc = x[:, c * T : (c + 1) * T, :]                       # [L, T, D]
        xt = xc.transpose(0, 2, 1).reshape(L, KD, 128, T)       # [L, kd, p, tok]
        xTp_cores.append(np.ascontiguousarray(xt.transpose(0, 2, 1, 3).reshape(L, 128, KD * T)))

    # weTp: [L, KD, 128, H]; [kd, p, h] = We[l, h, kd*128+p]
    weTp = np.ascontiguousarray(
        enc_weight.transpose(0, 2, 1).reshape(L, KD, 128, H)
    ).astype(np.float32)

    # wdTp: [NPAIRS, 128, NJ*O] bf16; [p, j*O+o] = Wd[s, t, o, j*128+p]
    wdTp = np.empty((NPAIRS, 128, NJ * O), dtype=bf)
    for i, (s, t) in enumerate(PAIRS):
        w = dec_weight[s, t]                                    # [O, H]
        wt = w.T.reshape(NJ, 128, O).transpose(1, 0, 2)         # [p, j, O]
        wdTp[i] = wt.reshape(128, NJ * O).astype(bf)

    return xTp_cores, weTp, wdTp


def kernel(layer_inputs, enc_weight, enc_bias, threshold, dec_weight, out_bias):
    from concourse.bass_utils import run_bass_kernel_spmd

    layer_inputs = np.asarray(layer_inputs, dtype=np.float32)
    enc_weight = np.asarray(enc_weight, dtype=np.float32)
    enc_bias = np.asarray(enc_bias, dtype=np.float32)
    threshold = np.asarray(threshold, dtype=np.float32)
    dec_weight = np.asarray(dec_weight, dtype=np.float32)
    out_bias = np.asarray(out_bias, dtype=np.float32)

    assert layer_inputs.shape == (L, B, S, D), layer_inputs.shape
    assert enc_weight.shape == (L, H, D), enc_weight.shape
    assert dec_weight.shape == (L, L, O, H), dec_weight.shape

    has_enc_bias = bool(np.any(enc_bias))
    has_threshold = bool(np.any(threshold))

    key = (has_enc_bias, has_threshold)
    if key not in _cache:
        _cache[key] = _build_program(*key)
    nc = _cache[key]

    xTp_cores, weTp, wdTp = _prep_inputs(
        layer_inputs, enc_weight, enc_bias, threshold, dec_weight
    )

    in_maps = []
    for c in range(NCORES):
        m = {"xTp": xTp_cores[c], "weTp": weTp, "wdTp": wdTp}
        if has_enc_bias:
            m["be"] = enc_bias
        if has_threshold:
            m["thr"] = threshold
        in_maps.append(m)

    res = run_bass_kernel_spmd(nc, in_maps, core_ids=list(range(NCORES)))
    if res.exec_time_ns is not None:
        print(f"HW exec time: {res.exec_time_ns} ns")

    out = np.empty((L, NTOK, O), dtype=np.float32)
    for c in range(NCORES):
        out[:, c * T : (c + 1) * T, :] = res.results[c]["out"]
    out += out_bias[:, None, :]
    return out.reshape(L, B, S, O)

